# revision 26
# baseline (speedup 1.0000x reference)
import numpy as np
import ml_dtypes

# nn_GRUDirectModel: 2-layer GRU (gate order r,z,n) + MLP head on 8 TRN2 cores.
# B=512, T=336, E=16, H=128, FH=24, FT=4.  Data-parallel: B/8=64 per core.
#
# Per-core layout: hidden state kept TRANSPOSED [H=128 partitions, B=64 cols]
# in bf16 so each recurrence matmul uses the (constant) weight matrix as the
# stationary operand.  All input projections are folded into the recurrence
# as accumulating matmuls (x is passed host-transposed [E+1, T*B] with a
# ones-row so L0 biases ride the same GEMM; L1 consumes h0 directly).
# Gates: sigmoid for r/z, tanh for n (one ACT table set), f32 gate math.
# The two per-layer chains run concurrently and interleave on ACT/DVE.

B, T, E, H, FH, FT = 512, 336, 16, 128, 24, 4
NCORES = 8
BC = B // NCORES  # 64 per-core batch

_COMPILED = {}

MERGED_RZ = False
DEPRI_OFF = 120
GATE_BUFS = 2
H_BUFS = 5
LAG = 2
PX_POS = 2
CHAIN_PRI = 0


def _patch_tile_drain():
    """walrus TPB_CTRL allows 1 sem wait; Tile's final drain can carry 3+.
    Split the drain's waits one-per-nop."""
    import concourse.tile as tile
    import concourse.mybir as mybir
    from concourse.vector_clock import ScopedClock

    if getattr(tile.TileContext, "_drain_split_patched", False):
        return

    def _drain_and_barrier(self, tick_clock, wait_clock):
        carrier = self.nc.sync.nop(nofuse=True, hint="drain_wait_split")
        wait_clock.add_sem_waits(
            carrier.ins, ScopedClock({None: tick_clock.global_clock})
        )
        si = carrier.ins.sync_info
        if si is not None and len(si.on_wait) > 1:
            waits = list(si.on_wait)
            carrier.ins.sync_info = mybir.SyncInfo(on_wait=waits[:1], on_update=[])
            for w in waits[1:]:
                c2 = self.nc.sync.nop(nofuse=True, hint="drain_wait_split")
                c2.ins.sync_info = mybir.SyncInfo(on_wait=[w], on_update=[])
        self.nc.sync.drain()
        self.nc.all_engine_barrier()
        popped = self.nc._tile_sem_poison_stack.pop()
        assert popped is self._sem_poison
        self.nc.clear_and_free_semaphores(list(self.sems.allocated().values()))
        self.nc.all_engine_barrier()

    tile.TileContext._drain_and_barrier = _drain_and_barrier
    tile.TileContext._drain_split_patched = True


def _install_wait_splitter():
    """This walrus build allows only ONE sync wait per instruction. After
    Tile assigns waits, split any instruction with >1 wait by inserting
    same-engine NoOp carriers directly before it (identical semantics)."""
    import concourse.bass as bass
    import orjson

    if getattr(bass.Bass, "_wait_split_patched", False):
        return
    orig = bass.Bass.to_json_bytes

    def to_json_bytes(self, *a, **k):
        bir = orig(self, *a, **k)
        d = orjson.loads(bir)
        changed = False
        for fn in d.get("functions", []):
            for bb in fn.get("blocks", []):
                insts = bb.get("instructions", [])
                out = []
                kctr = 0
                for inst in insts:
                    si = inst.get("sync_info")
                    ow = list(si.get("on_wait", [])) if si else []
                    if len(ow) > 1:
                        changed = True
                        for w in ow[:-1]:
                            kctr += 1
                            out.append({
                                "debug": inst.get("debug", 0),
                                "engine": inst["engine"],
                                "ins": [], "outs": [],
                                "name": f"{inst['name']}-ws{kctr}",
                                "opcode": "NoOp",
                                "text_hint": "wait_split",
                                "sync_info": {"on_update": [],
                                              "on_wait": [w]},
                            })
                        si["on_wait"] = [ow[-1]]
                    out.append(inst)
                bb["instructions"] = out
        return orjson.dumps(d) if changed else bir

    bass.Bass.to_json_bytes = to_json_bytes
    bass.Bass._wait_split_patched = True


def build_module(t_steps=T):
    """Build the Bass module (one SPMD core program)."""
    import concourse.bass as bass
    import concourse.mybir as mybir
    import concourse.tile as tile
    from contextlib import nullcontext

    _patch_tile_drain()
    _install_wait_splitter()
    dt = mybir.dt
    AF = mybir.ActivationFunctionType
    OP = mybir.AluOpType

    nc = bass.Bass("TRN2", target_bir_lowering=False, debug=False,
                   num_devices=NCORES)

    # ---- DRAM parameters (host-preprocessed, see kernel()) ----
    xa = nc.declare_dram_parameter("xa", [E + 1, t_steps * BC], dt.bfloat16,
                                   isOutput=False)
    xf = nc.declare_dram_parameter("xf", [FT + 1, BC * FH], dt.bfloat16,
                                   isOutput=False)
    wx0 = nc.declare_dram_parameter("wx0", [3, E + 1, H], dt.bfloat16,
                                    isOutput=False)
    wh0 = nc.declare_dram_parameter("wh0", [3, H, H], dt.bfloat16,
                                    isOutput=False)
    wx1 = nc.declare_dram_parameter("wx1", [3, H, H], dt.bfloat16,
                                    isOutput=False)
    wh1 = nc.declare_dram_parameter("wh1", [3, H, H], dt.bfloat16,
                                    isOutput=False)
    brz1 = nc.declare_dram_parameter("brz1", [2, 1, H], dt.bfloat16,
                                     isOutput=False)
    ones = nc.declare_dram_parameter("ones", [1, BC], dt.bfloat16,
                                     isOutput=False)
    bhn0 = nc.declare_dram_parameter("bhn0", [H, 1], dt.float32,
                                     isOutput=False)
    bhn1 = nc.declare_dram_parameter("bhn1", [H, 1], dt.float32,
                                     isOutput=False)
    bin1 = nc.declare_dram_parameter("bin1", [H, 1], dt.float32,
                                     isOutput=False)
    w1h = nc.declare_dram_parameter("w1h", [H, 128], dt.bfloat16,
                                    isOutput=False)
    w1f = nc.declare_dram_parameter("w1f", [FT + 1, 128], dt.bfloat16,
                                    isOutput=False)
    w2 = nc.declare_dram_parameter("w2", [128, 1], dt.bfloat16,
                                   isOutput=False)
    b2 = nc.declare_dram_parameter("b2", [1, 1], dt.float32, isOutput=False)
    out = nc.declare_dram_parameter("out", [1, BC * FH], dt.float32,
                                    isOutput=True)

    NC_MLP = 384  # MLP free-dim chunk: 16 batches x 24 horizons
    with tile.TileContext(nc) as tc:
        with (
            tc.tile_pool(name="const", bufs=1) as cpool,
            tc.tile_pool(name="state", bufs=H_BUFS) as hpool,
            tc.tile_pool(name="gates", bufs=GATE_BUFS) as gpool,
            tc.tile_pool(name="psum", bufs=3, space="PSUM") as ppool,
            tc.tile_pool(name="psumx", bufs=1, space="PSUM") as ppx,
            tc.tile_pool(name="mlp", bufs=1) as mpool,
        ):
            # ---- resident constants / inputs ----
            xa_s = cpool.tile([E + 1, t_steps * BC], dt.bfloat16)
            xa_chunks = 8 if t_steps >= 8 else 1
            xcw = (t_steps // xa_chunks) * BC
            for ci in range(xa_chunks):
                lo = ci * xcw
                hi = (ci + 1) * xcw if ci < xa_chunks - 1 else t_steps * BC
                nc.sync.dma_start(xa_s[:, lo:hi], xa[:, lo:hi])
            xf_s = cpool.tile([FT + 1, BC * FH], dt.bfloat16)
            nc.sync.dma_start(xf_s[:], xf[:])
            wx0_s = cpool.tile([E + 1, 3 * H], dt.bfloat16)
            wh0_s = cpool.tile([H, 3 * H], dt.bfloat16)
            wx1_s = cpool.tile([H, 3 * H], dt.bfloat16)
            wh1_s = cpool.tile([H, 3 * H], dt.bfloat16)
            for g in range(3):
                nc.sync.dma_start(wx0_s[:, g * H:(g + 1) * H], wx0[g])
                nc.sync.dma_start(wh0_s[:, g * H:(g + 1) * H], wh0[g])
                nc.sync.dma_start(wx1_s[:, g * H:(g + 1) * H], wx1[g])
                nc.sync.dma_start(wh1_s[:, g * H:(g + 1) * H], wh1[g])
            brz1_s = cpool.tile([1, 2 * H], dt.bfloat16)
            nc.sync.dma_start(brz1_s[:, 0:H], brz1[0])
            nc.sync.dma_start(brz1_s[:, H:2 * H], brz1[1])
            ones_s = cpool.tile([1, BC], dt.bfloat16)
            nc.sync.dma_start(ones_s[:], ones[:])
            bhn0_s = cpool.tile([H, 1], dt.float32)
            nc.sync.dma_start(bhn0_s[:], bhn0[:])
            bhn1_s = cpool.tile([H, 1], dt.float32)
            nc.sync.dma_start(bhn1_s[:], bhn1[:])
            bin1_s = cpool.tile([H, 1], dt.float32)
            nc.sync.dma_start(bin1_s[:], bin1[:])
            w1h_s = cpool.tile([H, 128], dt.bfloat16)
            nc.sync.dma_start(w1h_s[:], w1h[:])
            w1f_s = cpool.tile([FT + 1, 128], dt.bfloat16)
            nc.sync.dma_start(w1f_s[:], w1f[:])
            w2_s = cpool.tile([128, 1], dt.bfloat16)
            nc.sync.dma_start(w2_s[:], w2[:])
            b2_s = cpool.tile([1, 1], dt.float32)
            nc.sync.dma_start(b2_s[:], b2[:])

            # ---- const-warm dummies ----
            # Each engine touches every const tile once so later real ops'
            # waits on the const DMAs are elided (walrus allows at most 1
            # sync wait per instruction; extras split onto NoOps).
            scr = ppool.tile([H, 1], dt.float32, tag="ps0")
            pe_touch = [
                wx0_s[:, 0:H], wx0_s[:, H:2 * H], wx0_s[:, 2 * H:3 * H],
                wh0_s[:, 0:H], wh0_s[:, H:2 * H], wh0_s[:, 2 * H:3 * H],
                wx1_s[:, 0:H], wx1_s[:, H:2 * H], wx1_s[:, 2 * H:3 * H],
                wh1_s[:, 0:H], wh1_s[:, H:2 * H], wh1_s[:, 2 * H:3 * H],
                brz1_s[:, 0:H], brz1_s[:, H:2 * H], ones_s[:, 0:BC],
                xf_s[:, 0:128], w1f_s[:, 0:128], w1h_s[:, 0:H], w2_s[:, 0:1],
            ]
            for ap in pe_touch:
                m = ap.shape[1]
                nc.tensor.matmul(scr[0:m, 0:1], ap, ap[:, 0:1],
                                 start=True, stop=True)
            dvescr = gpool.tile([1, 8], dt.float32, tag="dvescr")
            for i, cst in enumerate((bhn0_s, bhn1_s, bin1_s, b2_s)):
                nc.vector.tensor_copy(dvescr[:, i:i + 1], cst[0:1, 0:1])

            # ---- initial hidden states (zero) ----
            h0 = hpool.tile([H, BC], dt.bfloat16, tag="h0")
            nc.vector.memset(h0[:], 0.0)
            h1 = hpool.tile([H, BC], dt.bfloat16, tag="h1")
            nc.vector.memset(h1[:], 0.0)

            def gru_step(t, h_prev, layer, x1=None, ones_t=None):
                """One GRU step in transposed layout; returns new h tile."""
                if layer == 0:
                    wx_s, wh_s, bhn_s = wx0_s, wh0_s, bhn0_s
                    xr = xa_s[:, t * BC:(t + 1) * BC]
                else:
                    wx_s, wh_s, bhn_s = wx1_s, wh1_s, bhn1_s
                    xr = x1
                # Separate PSUM tiles (= separate banks) so the critical
                # r->q->npre->n chain never serializes behind z's sigmoid
                # (Tile's bank-overlap tracker serializes same-bank pairs).
                ps = ppool.tile([H, 2 * BC], dt.float32, tag=f"ps{layer}")
                px = ppx.tile([H, 2 * BC], dt.float32, tag=f"px{layer}")
                # r/z: ps[:,0:64]=r_pre, ps[:,64:128]=z_pre
                for gi, base in ((0, 0), (1, BC)):
                    gsl = ps[:, base:base + BC]
                    if layer == 0:
                        nc.tensor.matmul(gsl, wx_s[:, gi * H:(gi + 1) * H],
                                         xr, start=True, stop=False)
                    else:
                        nc.tensor.matmul(gsl, brz1_s[:, gi * H:(gi + 1) * H],
                                         ones_t, start=True, stop=False)
                        nc.tensor.matmul(gsl, wx_s[:, gi * H:(gi + 1) * H],
                                         xr, start=False, stop=False)
                    nc.tensor.matmul(gsl, wh_s[:, gi * H:(gi + 1) * H],
                                     h_prev[:], start=False, stop=True)
                    if gi == 0 and PX_POS == 1:
                        nc.tensor.matmul(px[:, 0:BC], wh_s[:, 2 * H:3 * H],
                                         h_prev[:], start=True, stop=True)
                        nc.tensor.matmul(px[:, BC:2 * BC],
                                         wx_s[:, 2 * H:3 * H],
                                         xr, start=True, stop=True)
                if PX_POS == 2:
                    nc.tensor.matmul(px[:, 0:BC], wh_s[:, 2 * H:3 * H],
                                     h_prev[:], start=True, stop=True)
                    nc.tensor.matmul(px[:, BC:2 * BC], wx_s[:, 2 * H:3 * H],
                                     xr, start=True, stop=True)

                # off-chain copy hn|xpn psum -> sbuf (avoids the PSUM
                # access-latency ack on the on-chain q and npre ops)
                sx = gpool.tile([H, 2 * BC], dt.float32, tag=f"sx{layer}")
                nc.vector.tensor_copy(sx[:], px[:])
                r = gpool.tile([H, BC], dt.float32, tag=f"r{layer}")
                hp = (tc.high_priority(offset=CHAIN_PRI)
                      if CHAIN_PRI else nullcontext())
                with hp:
                    nc.scalar.activation(r[:], ps[:, 0:BC], AF.Sigmoid)
                dep = (tc.high_priority(offset=-DEPRI_OFF)
                       if DEPRI_OFF else nullcontext())
                with dep:
                    z = gpool.tile([H, BC], dt.float32, tag=f"z{layer}")
                    nc.scalar.activation(z[:], ps[:, BC:2 * BC], AF.Sigmoid)
                # off-chain (gpsimd): g = 1-z ; f = z*h   (h' = n*g + f)
                g = gpool.tile([H, BC], dt.float32, tag=f"g{layer}")
                nc.gpsimd.tensor_scalar(g[:], z[:], -1.0, 1.0,
                                        op0=OP.mult, op1=OP.add)
                fz = gpool.tile([H, BC], dt.float32, tag=f"f{layer}")
                nc.gpsimd.tensor_mul(fz[:], z[:], h_prev[:])
                # chain: q = (hn + b_hhn) * r ; npre = q + xpn ; n = tanh
                q = gpool.tile([H, BC], dt.float32, tag=f"q{layer}")
                nc.vector.scalar_tensor_tensor(
                    q[:], sx[:, 0:BC], bhn_s[:], r[:],
                    op0=OP.add, op1=OP.mult)
                npre = gpool.tile([H, BC], dt.float32, tag=f"np{layer}")
                if layer == 0:
                    nc.vector.tensor_add(npre[:], q[:], sx[:, BC:2 * BC])
                else:
                    nc.vector.scalar_tensor_tensor(
                        npre[:], sx[:, BC:2 * BC], bin1_s[:], q[:],
                        op0=OP.add, op1=OP.add)
                n = gpool.tile([H, BC], dt.float32, tag=f"n{layer}")
                hp2 = (tc.high_priority(offset=CHAIN_PRI)
                       if CHAIN_PRI else nullcontext())
                with hp2:
                    nc.scalar.activation(n[:], npre[:], AF.Tanh)
                # h' = n*g + f
                u = gpool.tile([H, BC], dt.float32, tag=f"u{layer}")
                nc.vector.tensor_mul(u[:], n[:], g[:])
                h_new = hpool.tile([H, BC], dt.bfloat16, tag=f"h{layer}")
                nc.vector.tensor_add(h_new[:], u[:], fz[:])
                return h_new, r

            # L1 lags L0 by LAG steps (its h0 input is then long ready);
            # L1's first r/z matmul streams a ones-vector regenerated from
            # L0's current-step r tile (still exactly 1.0), pinning L1's
            # chain phase midway through L0's so they don't collide on
            # ACT/DVE.
            h0hist = {}
            ints0 = None
            for t in range(t_steps + LAG):
                if t < t_steps:
                    h0, ints0 = gru_step(t, h0, 0)
                    h0hist[t] = h0
                if t >= LAG:
                    tl = t - LAG
                    if t < t_steps:
                        onest = gpool.tile([1, BC], dt.bfloat16, tag="onest")
                        nc.vector.tensor_scalar(onest[:], ints0[0:1, :],
                                                0.0, 1.0,
                                                op0=OP.mult, op1=OP.add)
                        ot = onest[:]
                    else:
                        ot = ones_s[:]
                    h1, _ = gru_step(tl, h1, 1, x1=h0hist.pop(tl)[:],
                                     ones_t=ot)

            # ---- MLP head ----
            # hid_pre[k,(b,f)] = W1f_aug @ xf_aug (+b1 via ones row)
            #                  + W1h @ h1 broadcast over f ; relu -> bf16
            hid = mpool.tile([128, BC * FH], dt.bfloat16)
            yout = mpool.tile([1, BC * FH], dt.float32)
            nb = NC_MLP // FH  # batches per chunk
            for c in range(BC * FH // NC_MLP):
                ps_b = ppool.tile([128, NC_MLP], dt.float32, tag="ps0")
                nc.tensor.matmul(ps_b[:], w1f_s[:],
                                 xf_s[:, c * NC_MLP:(c + 1) * NC_MLP],
                                 start=True, stop=False)
                h1b = (h1[:, c * nb:(c + 1) * nb]
                       .unsqueeze(-1).broadcast_to([H, nb, FH]))
                nc.tensor.matmul(ps_b[:], w1h_s[:], h1b,
                                 start=False, stop=True)
                nc.scalar.activation(hid[:, c * NC_MLP:(c + 1) * NC_MLP],
                                     ps_b[:], AF.Relu)
            for c in range(BC * FH // NC_MLP):
                ps_y = ppool.tile([1, NC_MLP], dt.float32, tag="ps1")
                nc.tensor.matmul(ps_y[:], w2_s[:],
                                 hid[:, c * NC_MLP:(c + 1) * NC_MLP],
                                 start=True, stop=True)
                nc.vector.tensor_scalar_add(
                    yout[:, c * NC_MLP:(c + 1) * NC_MLP], ps_y[:],
                    b2_s[0:1, 0:1])
            nc.sync.dma_start(out[:], yout[:])

    return nc


def _prep_inputs(x_enc, x_future_time,
                 W_ih0, W_hh0, b_ih0, b_hh0,
                 W_ih1, W_hh1, b_ih1, b_hh1,
                 W1, b1, W2, b2, t_steps=T):
    """Host-side: shard, transpose, augment, cast. Returns in_maps list."""
    bf16 = ml_dtypes.bfloat16
    f32 = np.float32
    x_enc = np.asarray(x_enc, f32)[:, :t_steps, :]
    x_future_time = np.asarray(x_future_time, f32)
    W_ih0, W_hh0, b_ih0, b_hh0 = [np.asarray(a, f32) for a in
                                  (W_ih0, W_hh0, b_ih0, b_hh0)]
    W_ih1, W_hh1, b_ih1, b_hh1 = [np.asarray(a, f32) for a in
                                  (W_ih1, W_hh1, b_ih1, b_hh1)]
    W1, b1, W2, b2 = [np.asarray(a, f32) for a in (W1, b1, W2, b2)]

    # L0 input weights augmented with combined r/z biases (n: b_ih only).
    wx0 = np.zeros((3, E + 1, H), f32)
    for g in range(3):
        wx0[g, :E, :] = W_ih0[g * H:(g + 1) * H, :].T
    wx0[0, E, :] = b_ih0[0:H] + b_hh0[0:H]
    wx0[1, E, :] = b_ih0[H:2 * H] + b_hh0[H:2 * H]
    wx0[2, E, :] = b_ih0[2 * H:3 * H]
    wh0 = np.stack([W_hh0[g * H:(g + 1) * H, :].T for g in range(3)])
    wx1 = np.stack([W_ih1[g * H:(g + 1) * H, :].T for g in range(3)])
    wh1 = np.stack([W_hh1[g * H:(g + 1) * H, :].T for g in range(3)])
    brz1 = np.stack([(b_ih1[0:H] + b_hh1[0:H])[None, :],
                     (b_ih1[H:2 * H] + b_hh1[H:2 * H])[None, :]])
    bhn0 = b_hh0[2 * H:3 * H].reshape(H, 1).copy()
    bhn1 = b_hh1[2 * H:3 * H].reshape(H, 1).copy()
    bin1 = b_ih1[2 * H:3 * H].reshape(H, 1).copy()
    # MLP: W1 = [128k, H+FT]; xf augmented with ones row carrying b1.
    w1h = W1[:, :H].T.copy()                      # [H, 128]
    w1f = np.zeros((FT + 1, 128), f32)
    w1f[:FT, :] = W1[:, H:].T
    w1f[FT, :] = b1
    w2m = W2.reshape(1, 128).T.copy()             # [128, 1]
    b2m = b2.reshape(1, 1).copy()

    common = dict(
        wx0=wx0.astype(bf16), wh0=wh0.astype(bf16),
        wx1=wx1.astype(bf16), wh1=wh1.astype(bf16),
        brz1=brz1.astype(bf16), ones=np.ones((1, BC), bf16),
        bhn0=bhn0, bhn1=bhn1, bin1=bin1,
        w1h=w1h.astype(bf16), w1f=w1f.astype(bf16),
        w2=w2m.astype(bf16), b2=b2m,
    )
    in_maps = []
    for c in range(NCORES):
        xs = x_enc[c * BC:(c + 1) * BC]            # [64, t, 16]
        xav = np.ones((E + 1, t_steps * BC), f32)
        xav[:E, :] = xs.transpose(2, 1, 0).reshape(E, t_steps * BC)
        xfs = x_future_time[c * BC:(c + 1) * BC]   # [64, 24, 4]
        xfv = np.ones((FT + 1, BC * FH), f32)
        xfv[:FT, :] = xfs.transpose(2, 0, 1).reshape(FT, BC * FH)
        m = dict(common)
        m["xa"] = xav.astype(bf16)
        m["xf"] = xfv.astype(bf16)
        in_maps.append(m)
    return in_maps


_RUNNER = None


def _get_runner(nc):
    """Build the sharded jit executor once; reuse across kernel() calls
    (rebuilding re-serializes the 50MB BIR per call)."""
    global _RUNNER
    if _RUNNER is not None:
        return _RUNNER
    import jax
    import numpy as _np
    from jax.sharding import Mesh, PartitionSpec
    from jax.experimental.shard_map import shard_map
    from concourse import bass2jax, mybir

    bass2jax.install_neuronx_cc_hook()
    part_name = (nc.partition_id_tensor.name
                 if nc.partition_id_tensor else None)
    in_names, out_names, out_avals, zero_shapes = [], [], [], []
    for alloc in nc.m.functions[0].allocations:
        if not isinstance(alloc, mybir.MemoryLocationSet):
            continue
        name = alloc.memorylocations[0].name
        if alloc.kind == "ExternalInput":
            if name != part_name:
                in_names.append(name)
        elif alloc.kind == "ExternalOutput":
            shape = tuple(alloc.tensor_shape)
            dtp = _np.dtype(mybir.dt.np(alloc.dtype))
            out_names.append(name)
            out_avals.append(jax.core.ShapedArray(shape, dtp))
            zero_shapes.append((shape, dtp))
    n_params = len(in_names)
    all_names = in_names + out_names
    if part_name is not None:
        all_names = all_names + [part_name]

    def _body(*args):
        operands = list(args)
        if part_name is not None:
            operands.append(bass2jax.partition_id_tensor())
        outs = bass2jax._bass_exec_p.bind(
            *operands,
            out_avals=tuple(out_avals),
            in_names=tuple(all_names),
            out_names=tuple(out_names),
            lowering_input_output_aliases=(),
            sim_require_finite=True,
            sim_require_nnan=True,
            nc=nc,
        )
        return tuple(outs)

    devices = jax.devices()[:NCORES]
    mesh = Mesh(_np.asarray(devices), ("core",))
    n_outs = len(out_names)
    sharded = jax.jit(
        shard_map(_body, mesh=mesh,
                  in_specs=(PartitionSpec("core"),) * (n_params + n_outs),
                  out_specs=(PartitionSpec("core"),) * n_outs,
                  check_rep=False),
        donate_argnums=tuple(range(n_params, n_params + n_outs)),
        keep_unused=True,
    )
    _RUNNER = (sharded, in_names, out_names, out_avals, zero_shapes, n_params)
    return _RUNNER


def kernel(x_enc, x_future_time,
           W_ih0, W_hh0, b_ih0, b_hh0,
           W_ih1, W_hh1, b_ih1, b_hh1,
           W1, b1, W2, b2):
    if T not in _COMPILED:
        _COMPILED[T] = build_module(T)
    nc = _COMPILED[T]
    in_maps = _prep_inputs(x_enc, x_future_time,
                           W_ih0, W_hh0, b_ih0, b_hh0,
                           W_ih1, W_hh1, b_ih1, b_hh1,
                           W1, b1, W2, b2)
    sharded, in_names, out_names, out_avals, zero_shapes, n_params = \
        _get_runner(nc)
    concat_in = [
        np.concatenate([np.asarray(in_maps[c][nm]) for c in range(NCORES)],
                       axis=0)
        for nm in in_names
    ]
    concat_zeros = [np.zeros((NCORES * sh[0], *sh[1:]), dtp)
                    for sh, dtp in zero_shapes]
    out_arrs = sharded(*concat_in, *concat_zeros)
    y = np.asarray(out_arrs[0]).reshape(NCORES, BC, FH)
    return y.reshape(B, FH).astype(np.float32)
